# revision 19
# baseline (speedup 1.0000x reference)
"""NeuralCDE RK4 solver as a Bass/Tile kernel on 8 Trainium2 cores.

Data-parallel over batch: B=1024 -> 128 rows per core (one partition tile).
Wall time = 508 serial RK4 stages x per-stage chain latency, so everything
here is about shortening that chain:
    kh    (DVE) : alpha*k^T   fp16 PSUM -> SBUF fp16 (2x mode)
    mm1acc (PE) : h_ps slot += W1zH.T @ kh    (big W1z.T @ zT pre-issued)
    relu  (DVE) : hS = relu(h_ps + bias1(t))
    mm2   (PE)  : f_ps = hS.T @ W2, split into two 256-col halves
    tanh  (ACT) : per half -> overlaps DVE mul/reduce of the other half
    mul   (DVE) : u = fS * g (broadcast over h), per half
    red   (DVE) : kn half = sum_c u
    T     (PE)  : kn^T -> ksP (fp16 non-accumulating, 1-pass)
RK4 sum: kn adds on DVE (off-chain); at s=3 the partial sum transposes
off-chain through a (1/6)-scaled identity, and one scalar_tensor_tensor
yields delta-z^T for both the state update and next step's h correction.
PE runs at the cold 1.2 GHz HAM rate (filler matmuls do not lift the clock
gate in this environment - verified experimentally).
"""

import numpy as np
import ml_dtypes

import concourse.bacc as bacc
import concourse.bass as bass
import concourse.mybir as mybir
from concourse.tile import TileContext
from concourse.bass_utils import run_bass_kernel_spmd

F32 = mybir.dt.float32
F32R = mybir.dt.float32r
BF16 = mybir.dt.bfloat16
FP16 = mybir.dt.float16
B = 1024
L = 128
C_IN = 8
HID = 64
MLP_H = 128
INIT_H = 20
NSTEP = L - 1  # 127
NCORES = 8
BL = B // NCORES  # 128 batch rows per core
NF = HID * C_IN  # 512
NH = NF // 2  # 256 (half of the f block, h-split)

_CACHE: dict = {}


def _flags():
    import os
    return (
        int(os.environ.get("F16_BIG", "0")),   # big mm1 in fp16 instead of f32r
        int(os.environ.get("SPLIT", "2")),     # 0 none, 1 sym halves, 2 asym 128/384
        int(os.environ.get("BIAS_MM", "1")),   # fold bias1 into h PSUM via rank-1 matmul
    )


def _build(nstep: int, with_b2: bool):
    import time as _time

    f16_big, split, bias_mm = _flags()
    BD = FP16 if f16_big else F32R
    t0 = _time.time()
    nc = bacc.Bacc()
    g_in = nc.dram_tensor("g", [BL, nstep * 3 * C_IN], FP16, kind="ExternalInput")
    b1_in = nc.dram_tensor("bias1", [MLP_H, nstep * 3], F32, kind="ExternalInput")
    b1t_in = nc.dram_tensor("bias1t", [1, nstep * 3 * MLP_H], F32, kind="ExternalInput")
    ones32_in = nc.dram_tensor("ones32", [1, BL], F32, kind="ExternalInput")
    w1z_in = nc.dram_tensor("w1z", [HID, MLP_H], BD, kind="ExternalInput")
    w1zh_in = nc.dram_tensor("w1zh", [HID, MLP_H], FP16, kind="ExternalInput")
    w2_in = nc.dram_tensor("w2", [MLP_H, NF], FP16, kind="ExternalInput")
    b2_in = nc.dram_tensor("b2r", [1, NF], FP16, kind="ExternalInput")
    ones_in = nc.dram_tensor("onesr", [1, BL], FP16, kind="ExternalInput")
    id_in = nc.dram_tensor("ident", [BL, BL], FP16, kind="ExternalInput")
    id6_in = nc.dram_tensor("ident6", [BL, BL], FP16, kind="ExternalInput")
    z0t_in = nc.dram_tensor("z0t", [HID, BL], F32R, kind="ExternalInput")
    zs_out = nc.dram_tensor("zs", [HID, (nstep + 1) * BL], F32, kind="ExternalOutput")

    with TileContext(nc) as tc:
        with (
            tc.tile_pool(name="const", bufs=1) as cp,
            tc.tile_pool(name="zst", bufs=1) as zp,
            tc.tile_pool(name="hs", bufs=3) as hp,
            tc.tile_pool(name="fs", bufs=3) as fp,
            tc.tile_pool(name="us", bufs=3) as up,
            tc.tile_pool(name="ks", bufs=3) as kp,
            tc.tile_pool(name="an", bufs=2) as ap,
            tc.tile_pool(name="zs2", bufs=2) as zsp,
            tc.tile_pool(name="bt", bufs=2) as btp,
            tc.tile_pool(name="kh", bufs=2) as khp,
            tc.tile_pool(name="ph", bufs=1, space="PSUM") as ph,
            tc.tile_pool(name="pf", bufs=2, space="PSUM") as pf,
            tc.tile_pool(name="pkt", bufs=3, space="PSUM") as pkt,
            tc.tile_pool(name="pa26", bufs=1, space="PSUM") as pa26,
        ):
            b1S = cp.tile([MLP_H, nstep * 3], F32)
            gS = cp.tile([BL, nstep * 3 * C_IN], FP16)
            w1zS = cp.tile([HID, MLP_H], BD)
            w1zH = cp.tile([HID, MLP_H], FP16)
            w2S = cp.tile([MLP_H, NF], FP16)
            b2S = cp.tile([1, NF], FP16)
            onesS = cp.tile([1, BL], FP16)
            idS = cp.tile([BL, BL], FP16)
            ones32S = cp.tile([1, BL], F32)
            id6S = cp.tile([BL, BL], FP16)
            zall = zp.tile([HID, (nstep + 1) * BL], F32R)

            nc.sync.dma_start(out=gS[:], in_=g_in[:])
            nc.sync.dma_start(out=b1S[:], in_=b1_in[:])
            nc.sync.dma_start(out=w1zS[:], in_=w1z_in[:])
            nc.sync.dma_start(out=w1zH[:], in_=w1zh_in[:])
            nc.sync.dma_start(out=w2S[:], in_=w2_in[:])
            nc.sync.dma_start(out=b2S[:], in_=b2_in[:])
            nc.sync.dma_start(out=onesS[:], in_=ones_in[:])
            nc.sync.dma_start(out=idS[:], in_=id_in[:])
            nc.sync.dma_start(out=ones32S[:], in_=ones32_in[:])
            nc.sync.dma_start(out=id6S[:], in_=id6_in[:])
            nc.sync.dma_start(out=zall[:, 0:BL], in_=z0t_in[:])
            nc.sync.dma_start(out=zs_out[:, 0:BL], in_=z0t_in[:].bitcast(F32))

            # h PSUM: one bank, 4 rotating [128,128] stage slots. Emission
            # order guarantees at most one open accumulation group at a time.
            hP = ph.tile([MLP_H, 4 * BL], F32, name="hP")

            def h_slot(step, s):
                i = (4 * step + s) % 4
                return hP[:, i * BL : (i + 1) * BL]

            CLS = (0, 1, 1, 2)
            KH_A = (1.0 / 6.0, 0.5, 0.25, 0.5)

            def zT_sl(step):
                return zall[:, step * BL : (step + 1) * BL]

            def bt_tile(step):
                t = btp.tile([1, 3 * MLP_H], F32, tag="bt", name="bt")
                nc.sync.dma_start(
                    out=t[:],
                    in_=b1t_in[:, step * 3 * MLP_H : (step + 1) * 3 * MLP_H],
                )
                return t

            bt_cur = bt_tile(0) if bias_mm else None
            bt_next = None

            # step 0 slice-0 big (no k correction at the very first stage)
            nc.tensor.matmul(
                h_slot(0, 0), lhsT=w1zS[:], rhs=zT_sl(0), start=True,
                stop=not bias_mm,
            )
            if bias_mm:
                nc.tensor.matmul(
                    h_slot(0, 0), lhsT=bt_cur[:, 0:MLP_H], rhs=ones32S[:],
                    start=False, stop=True, skip_group_check=True,
                )

            acc_nat = None   # kn1+kn2 (+kn3) natural-layout partial RK4 sum
            acc2T6 = None    # (acc_nat at s=2).T / 6 in PSUM
            zsum = None      # zT + acc2T6, f32r (state update staging)
            kt4P = None      # k4~.T PSUM
            ksP = None       # k~_s.T PSUM for next stage's kh
            kh0 = None       # delta-z^T fp16 (next step's h correction)

            for step in range(nstep):
                zT = zT_sl(step)
                if bias_mm and step + 1 < nstep:
                    bt_next = bt_tile(step + 1)
                for s in range(4):
                    col = step * 3 + CLS[s]
                    has_b = not (step == 0 and s == 0)
                    # ---- kh for this stage ----
                    if has_b:
                        kh = khp.tile([HID, BL], FP16, tag="kh", name="kh")
                        if s == 0:
                            # kh0 = (k4~.T)/6 + acc2T6 = delta-z^T
                            nc.vector.scalar_tensor_tensor(
                                out=kh[:],
                                in0=kt4P[:],
                                scalar=1.0 / 6.0,
                                in1=acc2T6[:],
                                op0=mybir.AluOpType.mult,
                                op1=mybir.AluOpType.add,
                            )
                            # state update z_step = zsum + (k4~.T)/6, f32r
                            nc.vector.scalar_tensor_tensor(
                                out=zT,
                                in0=kt4P[:],
                                scalar=1.0 / 6.0,
                                in1=zsum[:],
                                op0=mybir.AluOpType.mult,
                                op1=mybir.AluOpType.add,
                            )
                            nc.sync.dma_start(
                                out=zs_out[:, step * BL : (step + 1) * BL],
                                in_=zT.bitcast(F32),
                            )
                        else:
                            nc.vector.tensor_scalar_mul(kh[:], ksP[:], KH_A[s])
                        nc.tensor.matmul(
                            h_slot(step, s), lhsT=w1zH[:], rhs=kh[:],
                            start=False, stop=True,
                        )
                    # ---- big mm1 for the next stage slot (off chain) ----
                    if s < 3:
                        nc.tensor.matmul(
                            h_slot(step, s + 1), lhsT=w1zS[:], rhs=zT,
                            start=True, stop=False,
                        )
                        if bias_mm:
                            ncol = CLS[s + 1]
                            nc.tensor.matmul(
                                h_slot(step, s + 1),
                                lhsT=bt_cur[:, ncol * MLP_H : (ncol + 1) * MLP_H],
                                rhs=ones32S[:],
                                start=False, stop=False, skip_group_check=True,
                            )
                    elif step + 1 < nstep:
                        nc.tensor.matmul(
                            h_slot(step + 1, 0), lhsT=w1zS[:], rhs=zT,
                            start=True, stop=False,
                        )
                        if bias_mm:
                            nc.tensor.matmul(
                                h_slot(step + 1, 0),
                                lhsT=bt_next[:, 0:MLP_H],
                                rhs=ones32S[:],
                                start=False, stop=False, skip_group_check=True,
                            )
                    # ---- relu (bias already in PSUM when bias_mm) ----
                    hS = hp.tile([MLP_H, BL], FP16, tag="hs")
                    if bias_mm:
                        nc.vector.tensor_scalar_max(hS[:], h_slot(step, s), 0.0)
                    else:
                        nc.vector.tensor_scalar(
                            hS[:], h_slot(step, s), b1S[:, col : col + 1], 0.0,
                            op0=mybir.AluOpType.add, op1=mybir.AluOpType.max,
                        )
                    # ---- mm2 (+ optional bias2), h-split halves ----
                    f_ps = pf.tile([BL, NF], F32, tag="fps")
                    if with_b2:
                        nc.tensor.matmul(
                            f_ps[:], lhsT=onesS[:], rhs=b2S[:],
                            start=True, stop=False,
                        )
                    if split == 2:
                        halves = ((0, 128), (128, NF))
                    elif split == 1:
                        halves = ((0, NH), (NH, NF))
                    else:
                        halves = ((0, NF),)
                    for (lo, hi) in halves:
                        nc.tensor.matmul(
                            f_ps[:, lo:hi], lhsT=hS[:], rhs=w2S[:, lo:hi],
                            start=not with_b2, stop=True,
                        )
                    # ---- tanh / mul / reduce, pipelined across halves ----
                    fS = fp.tile([BL, NF], FP16, tag="fs")
                    u = up.tile([BL, NF], FP16, tag="u")
                    kn = kp.tile([BL, HID], FP16, tag="kn")
                    for (lo, hi) in halves:
                        nc.scalar.activation(
                            fS[:, lo:hi], f_ps[:, lo:hi],
                            mybir.ActivationFunctionType.Tanh,
                        )
                    for (lo, hi) in halves:
                        hlo, hhi = lo // C_IN, hi // C_IN
                        f3 = fS[:, lo:hi].rearrange("p (h c) -> p h c", c=C_IN)
                        u3 = u[:, lo:hi].rearrange("p (h c) -> p h c", c=C_IN)
                        gv = (
                            gS[:, col * C_IN : (col + 1) * C_IN]
                            .unsqueeze(1)
                            .broadcast_to((BL, hhi - hlo, C_IN))
                        )
                        nc.vector.tensor_tensor(
                            out=u3, in0=f3, in1=gv, op=mybir.AluOpType.mult
                        )
                        with nc.allow_low_precision("k reduce"):
                            nc.vector.tensor_reduce(
                                kn[:, hlo:hhi], u3, axis=mybir.AxisListType.X,
                                op=mybir.AluOpType.add,
                            )
                    # ---- transpose + RK4 bookkeeping ----
                    if s < 3:
                        ksP = pkt.tile([HID, BL], FP16, tag="kt", name="kt")
                        nc.tensor.matmul(
                            ksP[:], lhsT=kn[:], rhs=idS[:], is_transpose=True,
                            start=True, stop=True,
                        )
                        if s == 0:
                            acc_nat = kn
                        else:
                            acc_new = ap.tile([BL, HID], FP16, tag="an",
                                              name="an")
                            nc.vector.tensor_tensor(
                                out=acc_new[:], in0=acc_nat[:], in1=kn[:],
                                op=mybir.AluOpType.add,
                            )
                            acc_nat = acc_new
                        if s == 2:
                            # off-chain: acc2T6 = acc_nat.T / 6 (via scaled
                            # identity), staged to SBUF so the s=0 STT has a
                            # single PSUM operand; zsum = zT + acc2T6
                            a26P = pa26.tile([HID, BL], F32, tag="a26",
                                             name="a26")
                            # regular matmul: transpose datapath would ignore
                            # the scaled identity's values
                            nc.tensor.matmul(
                                a26P[:], lhsT=acc_nat[:], rhs=id6S[:],
                                start=True, stop=True,
                            )
                            acc2T6 = zsp.tile([HID, BL], FP16, tag="a26s",
                                              name="a26s")
                            nc.vector.tensor_copy(acc2T6[:], a26P[:])
                            zsum = zsp.tile([HID, BL], F32R, tag="zsum",
                                            name="zsum")
                            nc.vector.tensor_tensor(
                                out=zsum[:], in0=zT, in1=acc2T6[:],
                                op=mybir.AluOpType.add,
                            )
                    else:
                        kt4P = pkt.tile([HID, BL], FP16, tag="kt", name="kt4")
                        nc.tensor.matmul(
                            kt4P[:], lhsT=kn[:], rhs=idS[:], is_transpose=True,
                            start=True, stop=True,
                        )
                        bt_cur = bt_next

            # final z update (last grid point)
            nc.vector.scalar_tensor_tensor(
                out=zT_sl(nstep),
                in0=kt4P[:],
                scalar=1.0 / 6.0,
                in1=zsum[:],
                op0=mybir.AluOpType.mult,
                op1=mybir.AluOpType.add,
            )
            nc.sync.dma_start(
                out=zs_out[:, nstep * BL : (nstep + 1) * BL],
                in_=zT_sl(nstep).bitcast(F32),
            )
    import sys

    print(f"[kernel] tile trace+schedule: {_time.time()-t0:.1f}s", file=sys.stderr)
    t1 = _time.time()
    nc.finalize()
    print(f"[kernel] finalize: {_time.time()-t1:.1f}s", file=sys.stderr)
    return nc


def _get_nc(nstep: int, with_b2: bool):
    key = (nstep, with_b2) + _flags()
    if key not in _CACHE:
        _CACHE[key] = _build(nstep, with_b2)
    return _CACHE[key]


def _host_prep(coeffs, Wi1, bi1, Wi2, bi2, W1, b1, W2, b2, nstep: int):
    coeffs = np.asarray(coeffs, dtype=np.float32)
    a = coeffs[:, :, 0:8]
    b = coeffs[:, :, 8:16]
    c = coeffs[:, :, 16:24]
    d = coeffs[:, :, 24:32]

    X0 = a[:, 0]
    z0 = np.tanh(
        np.maximum(X0 @ Wi1 + bi1, 0.0).astype(np.float32) @ Wi2 + bi2
    ).astype(np.float32)

    g = np.empty((B, nstep, 3, C_IN), dtype=np.float32)
    g[:, :, 0] = b[:, :nstep]
    g[:, :, 1] = 2.0 * b[:, :nstep] + 2.0 * c[:, :nstep] + 1.5 * d[:, :nstep]
    last = NSTEP - 1  # 126 in full problem
    for i in range(nstep):
        if i < last:
            g[:, i, 2] = b[:, i + 1]
        else:
            g[:, i, 2] = b[:, i] + 2.0 * c[:, i] + 3.0 * d[:, i]
    g16 = g.reshape(B, nstep * 3 * C_IN).astype(np.float16)

    tcols = np.empty((nstep, 3), dtype=np.float32)
    tcols[:, 0] = np.arange(nstep, dtype=np.float32)
    tcols[:, 1] = tcols[:, 0] + 0.5
    tcols[:, 2] = tcols[:, 0] + 1.0
    bias1 = (
        b1[None, None, :] + tcols[:, :, None] * W1[0][None, None, :]
    ).astype(np.float32)
    bias1 = bias1.reshape(nstep * 3, MLP_H).T.copy()  # [128, nstep*3]

    f16_big = _flags()[0]
    bias1t = np.ascontiguousarray(bias1.T.reshape(1, -1))
    shared = {
        "bias1": bias1,
        "bias1t": bias1t,
        "ones32": np.ones((1, BL), dtype=np.float32),
        "w1z": np.ascontiguousarray(
            W1[1:], dtype=(np.float16 if f16_big else np.float32)
        ),
        "w1zh": np.ascontiguousarray(W1[1:], dtype=np.float16),
        "w2": np.ascontiguousarray(W2, dtype=np.float16),
        "b2r": np.ascontiguousarray(b2[None, :], dtype=np.float16),
        "onesr": np.ones((1, BL), dtype=np.float16),
        "ident": np.eye(BL, dtype=np.float16),
        "ident6": (np.eye(BL) / 6.0).astype(np.float16),
    }
    in_maps = []
    for core in range(NCORES):
        sl = slice(core * BL, (core + 1) * BL)
        m = dict(shared)
        m["g"] = np.ascontiguousarray(g16[sl])
        m["z0t"] = np.ascontiguousarray(z0[sl].T)
        in_maps.append(m)
    return in_maps, z0


def kernel(coeffs, Wi1, bi1, Wi2, bi2, W1, b1, W2, b2, _nstep: int = NSTEP,
           _trace: bool = False):
    import time as _time
    import sys

    nstep = _nstep
    with_b2 = bool(np.any(np.asarray(b2)))
    nc = _get_nc(nstep, with_b2)
    in_maps, _ = _host_prep(
        coeffs, Wi1, bi1, Wi2, bi2, W1, b1, W2, b2, nstep
    )
    t0 = _time.time()
    res = run_bass_kernel_spmd(nc, in_maps, list(range(NCORES)), trace=_trace)
    print(f"[kernel] spmd run (compile+exec): {_time.time()-t0:.1f}s", file=sys.stderr)
    out = np.empty((B, nstep + 1, HID), dtype=np.float32)
    for core in range(NCORES):
        zs = res.results[core]["zs"].reshape(HID, nstep + 1, BL)
        out[core * BL : (core + 1) * BL] = zs.transpose(2, 1, 0)
    if _trace:
        kernel.last_results = res
    return out


# revision 20
# speedup vs baseline: 1.2701x; 1.2701x over previous
"""NeuralCDE RK4 solver as a Bass/Tile kernel on 8 Trainium2 cores.

Data-parallel over batch: B=1024 -> 128 rows per core (one partition tile).
Wall time = 508 serial RK4 stages x per-stage chain latency, so everything
here is about shortening that chain:
    kh    (DVE) : alpha*k^T   fp16 PSUM -> SBUF fp16 (2x mode)
    mm1acc (PE) : h_ps slot += W1zH.T @ kh    (big W1z.T @ zT pre-issued)
    relu  (DVE) : hS = relu(h_ps + bias1(t))
    mm2   (PE)  : f_ps = hS.T @ W2, split into two 256-col halves
    tanh  (ACT) : per half -> overlaps DVE mul/reduce of the other half
    mul   (DVE) : u = fS * g (broadcast over h), per half
    red   (DVE) : kn half = sum_c u
    T     (PE)  : kn^T -> ksP (fp16 non-accumulating, 1-pass)
RK4 sum: kn adds on DVE (off-chain); at s=3 the partial sum transposes
off-chain through a (1/6)-scaled identity, and one scalar_tensor_tensor
yields delta-z^T for both the state update and next step's h correction.
PE runs at the cold 1.2 GHz HAM rate (filler matmuls do not lift the clock
gate in this environment - verified experimentally).
"""

import numpy as np
import ml_dtypes

import concourse.bacc as bacc
import concourse.bass as bass
import concourse.mybir as mybir
from concourse.tile import TileContext
from concourse.bass_utils import run_bass_kernel_spmd

F32 = mybir.dt.float32
F32R = mybir.dt.float32r
BF16 = mybir.dt.bfloat16
FP16 = mybir.dt.float16
B = 1024
L = 128
C_IN = 8
HID = 64
MLP_H = 128
INIT_H = 20
NSTEP = L - 1  # 127
NCORES = 8
BL = B // NCORES  # 128 batch rows per core
NF = HID * C_IN  # 512
NH = NF // 2  # 256 (half of the f block, h-split)

_CACHE: dict = {}


def _flags():
    import os
    return (
        int(os.environ.get("F16_BIG", "0")),   # big mm1 in fp16 instead of f32r
        int(os.environ.get("SPLIT", "2")),     # 0 none, 1 sym halves, 2 asym 128/384
        int(os.environ.get("BIAS_MM", "1")),   # fold bias1 into h PSUM via rank-1 matmul
    )


def _build(nstep: int, with_b2: bool):
    import time as _time

    f16_big, split, bias_mm = _flags()
    BD = FP16 if f16_big else F32R
    t0 = _time.time()
    nc = bacc.Bacc()
    g_in = nc.dram_tensor("g", [BL, nstep * 3 * C_IN], FP16, kind="ExternalInput")
    b1_in = nc.dram_tensor("bias1", [MLP_H, nstep * 3], F32, kind="ExternalInput")
    b1t_in = nc.dram_tensor("bias1t", [1, nstep * 3 * MLP_H], F32, kind="ExternalInput")
    ones32_in = nc.dram_tensor("ones32", [1, BL], F32, kind="ExternalInput")
    w1z_in = nc.dram_tensor("w1z", [HID, MLP_H], BD, kind="ExternalInput")
    w1zh_in = nc.dram_tensor("w1zh", [HID, MLP_H], FP16, kind="ExternalInput")
    w2_in = nc.dram_tensor("w2", [MLP_H, NF], FP16, kind="ExternalInput")
    b2_in = nc.dram_tensor("b2r", [1, NF], FP16, kind="ExternalInput")
    ones_in = nc.dram_tensor("onesr", [1, BL], FP16, kind="ExternalInput")
    id_in = nc.dram_tensor("ident", [BL, BL], FP16, kind="ExternalInput")
    id6_in = nc.dram_tensor("ident6", [BL, BL], FP16, kind="ExternalInput")
    z0t_in = nc.dram_tensor("z0t", [HID, BL], F32R, kind="ExternalInput")
    zs_out = nc.dram_tensor("zs", [HID, (nstep + 1) * BL], F32, kind="ExternalOutput")

    with TileContext(nc) as tc:
        with (
            tc.tile_pool(name="const", bufs=1) as cp,
            tc.tile_pool(name="zst", bufs=1) as zp,
            tc.tile_pool(name="hs", bufs=3) as hp,
            tc.tile_pool(name="fs", bufs=3) as fp,
            tc.tile_pool(name="us", bufs=3) as up,
            tc.tile_pool(name="ks", bufs=3) as kp,
            tc.tile_pool(name="an", bufs=2) as ap,
            tc.tile_pool(name="zs2", bufs=2) as zsp,
            tc.tile_pool(name="bt", bufs=2) as btp,
            tc.tile_pool(name="kh", bufs=2) as khp,
            tc.tile_pool(name="ph", bufs=1, space="PSUM") as ph,
            tc.tile_pool(name="pf", bufs=2, space="PSUM") as pf,
            tc.tile_pool(name="pkt", bufs=3, space="PSUM") as pkt,
            tc.tile_pool(name="pa26", bufs=1, space="PSUM") as pa26,
        ):
            b1S = cp.tile([MLP_H, nstep * 3], F32)
            gS = cp.tile([BL, nstep * 3 * C_IN], FP16)
            w1zS = cp.tile([HID, MLP_H], BD)
            w1zH = cp.tile([HID, MLP_H], FP16)
            w2S = cp.tile([MLP_H, NF], FP16)
            b2S = cp.tile([1, NF], FP16)
            onesS = cp.tile([1, BL], FP16)
            idS = cp.tile([BL, BL], FP16)
            ones32S = cp.tile([1, BL], F32)
            id6S = cp.tile([BL, BL], FP16)
            zall = zp.tile([HID, (nstep + 1) * BL], F32R)

            nc.sync.dma_start(out=gS[:], in_=g_in[:])
            nc.sync.dma_start(out=b1S[:], in_=b1_in[:])
            nc.sync.dma_start(out=w1zS[:], in_=w1z_in[:])
            nc.sync.dma_start(out=w1zH[:], in_=w1zh_in[:])
            nc.sync.dma_start(out=w2S[:], in_=w2_in[:])
            nc.sync.dma_start(out=b2S[:], in_=b2_in[:])
            nc.sync.dma_start(out=onesS[:], in_=ones_in[:])
            nc.sync.dma_start(out=idS[:], in_=id_in[:])
            nc.sync.dma_start(out=ones32S[:], in_=ones32_in[:])
            nc.sync.dma_start(out=id6S[:], in_=id6_in[:])
            nc.sync.dma_start(out=zall[:, 0:BL], in_=z0t_in[:])
            nc.sync.dma_start(out=zs_out[:, 0:BL], in_=z0t_in[:].bitcast(F32))

            # h PSUM: one bank, 4 rotating [128,128] stage slots. Emission
            # order guarantees at most one open accumulation group at a time.
            hP = ph.tile([MLP_H, 4 * BL], F32, name="hP")

            def h_slot(step, s):
                i = (4 * step + s) % 4
                return hP[:, i * BL : (i + 1) * BL]

            CLS = (0, 1, 1, 2)
            KH_A = (1.0 / 6.0, 0.5, 0.25, 0.5)

            def zT_sl(step):
                return zall[:, step * BL : (step + 1) * BL]

            def bt_tile(step):
                t = btp.tile([1, 3 * MLP_H], F32, tag="bt", name="bt")
                nc.sync.dma_start(
                    out=t[:],
                    in_=b1t_in[:, step * 3 * MLP_H : (step + 1) * 3 * MLP_H],
                )
                return t

            bt_cur = bt_tile(0) if bias_mm else None
            bt_next = None

            # step 0 slice-0 big (no k correction at the very first stage)
            nc.tensor.matmul(
                h_slot(0, 0), lhsT=w1zS[:], rhs=zT_sl(0), start=True,
                stop=not bias_mm,
            )
            if bias_mm:
                nc.tensor.matmul(
                    h_slot(0, 0), lhsT=bt_cur[:, 0:MLP_H], rhs=ones32S[:],
                    start=False, stop=True, skip_group_check=True,
                )

            acc_nat = None   # kn1+kn2 (+kn3) natural-layout partial RK4 sum
            acc2T6 = None    # (acc_nat at s=2).T / 6 in PSUM
            zsum = None      # zT + acc2T6, f32r (state update staging)
            kt4P = None      # k4~.T PSUM
            ksP = None       # k~_s.T PSUM for next stage's kh
            kh0 = None       # delta-z^T fp16 (next step's h correction)

            for step in range(nstep):
                zT = zT_sl(step)
                if bias_mm and step + 1 < nstep:
                    bt_next = bt_tile(step + 1)
                for s in range(4):
                    col = step * 3 + CLS[s]
                    has_b = not (step == 0 and s == 0)
                    # ---- kh for this stage ----
                    if has_b:
                        kh = khp.tile([HID, BL], FP16, tag="kh", name="kh")
                        if s == 0:
                            # kh0 = (k4~.T)/6 + acc2T6 = delta-z^T
                            nc.vector.scalar_tensor_tensor(
                                out=kh[:],
                                in0=kt4P[:],
                                scalar=1.0 / 6.0,
                                in1=acc2T6[:],
                                op0=mybir.AluOpType.mult,
                                op1=mybir.AluOpType.add,
                            )
                            # state update z_step = zsum + (k4~.T)/6, f32r
                            nc.vector.scalar_tensor_tensor(
                                out=zT,
                                in0=kt4P[:],
                                scalar=1.0 / 6.0,
                                in1=zsum[:],
                                op0=mybir.AluOpType.mult,
                                op1=mybir.AluOpType.add,
                            )
                            nc.sync.dma_start(
                                out=zs_out[:, step * BL : (step + 1) * BL],
                                in_=zT.bitcast(F32),
                            )
                        else:
                            nc.vector.tensor_scalar_mul(kh[:], ksP[:], KH_A[s])
                        nc.tensor.matmul(
                            h_slot(step, s), lhsT=w1zH[:], rhs=kh[:],
                            start=False, stop=True,
                        )
                    # ---- relu (bias already in PSUM when bias_mm) ----
                    hS = hp.tile([MLP_H, BL], FP16, tag="hs")
                    if bias_mm:
                        nc.vector.tensor_scalar_max(hS[:], h_slot(step, s), 0.0)
                    else:
                        nc.vector.tensor_scalar(
                            hS[:], h_slot(step, s), b1S[:, col : col + 1], 0.0,
                            op0=mybir.AluOpType.add, op1=mybir.AluOpType.max,
                        )
                    # ---- mm2 (+ optional bias2), h-split halves ----
                    f_ps = pf.tile([BL, NF], F32, tag="fps")
                    if with_b2:
                        nc.tensor.matmul(
                            f_ps[:], lhsT=onesS[:], rhs=b2S[:],
                            start=True, stop=False,
                        )
                    if split == 2:
                        halves = ((0, 128), (128, NF))
                    elif split == 1:
                        halves = ((0, NH), (NH, NF))
                    else:
                        halves = ((0, NF),)
                    for (lo, hi) in halves:
                        nc.tensor.matmul(
                            f_ps[:, lo:hi], lhsT=hS[:], rhs=w2S[:, lo:hi],
                            start=not with_b2, stop=True,
                        )
                    # ---- big mm1 for the next stage slot (off chain) ----
                    if s < 3:
                        nc.tensor.matmul(
                            h_slot(step, s + 1), lhsT=w1zS[:], rhs=zT,
                            start=True, stop=False,
                        )
                        if bias_mm:
                            ncol = CLS[s + 1]
                            nc.tensor.matmul(
                                h_slot(step, s + 1),
                                lhsT=bt_cur[:, ncol * MLP_H : (ncol + 1) * MLP_H],
                                rhs=ones32S[:],
                                start=False, stop=False, skip_group_check=True,
                            )
                    elif step + 1 < nstep:
                        nc.tensor.matmul(
                            h_slot(step + 1, 0), lhsT=w1zS[:], rhs=zT,
                            start=True, stop=False,
                        )
                        if bias_mm:
                            nc.tensor.matmul(
                                h_slot(step + 1, 0),
                                lhsT=bt_next[:, 0:MLP_H],
                                rhs=ones32S[:],
                                start=False, stop=False, skip_group_check=True,
                            )
                    # ---- tanh / mul / reduce, pipelined across halves ----
                    fS = fp.tile([BL, NF], FP16, tag="fs")
                    u = up.tile([BL, NF], FP16, tag="u")
                    kn = kp.tile([BL, HID], FP16, tag="kn")
                    for (lo, hi) in halves:
                        nc.scalar.activation(
                            fS[:, lo:hi], f_ps[:, lo:hi],
                            mybir.ActivationFunctionType.Tanh,
                        )
                    for (lo, hi) in halves:
                        hlo, hhi = lo // C_IN, hi // C_IN
                        f3 = fS[:, lo:hi].rearrange("p (h c) -> p h c", c=C_IN)
                        u3 = u[:, lo:hi].rearrange("p (h c) -> p h c", c=C_IN)
                        gv = (
                            gS[:, col * C_IN : (col + 1) * C_IN]
                            .unsqueeze(1)
                            .broadcast_to((BL, hhi - hlo, C_IN))
                        )
                        nc.vector.tensor_tensor(
                            out=u3, in0=f3, in1=gv, op=mybir.AluOpType.mult
                        )
                        with nc.allow_low_precision("k reduce"):
                            nc.vector.tensor_reduce(
                                kn[:, hlo:hhi], u3, axis=mybir.AxisListType.X,
                                op=mybir.AluOpType.add,
                            )
                    # ---- transpose + RK4 bookkeeping ----
                    if s < 3:
                        ksP = pkt.tile([HID, BL], FP16, tag="kt", name="kt")
                        nc.tensor.matmul(
                            ksP[:], lhsT=kn[:], rhs=idS[:], is_transpose=True,
                            start=True, stop=True,
                        )
                        if s == 0:
                            acc_nat = kn
                        else:
                            acc_new = ap.tile([BL, HID], FP16, tag="an",
                                              name="an")
                            nc.vector.tensor_tensor(
                                out=acc_new[:], in0=acc_nat[:], in1=kn[:],
                                op=mybir.AluOpType.add,
                            )
                            acc_nat = acc_new
                        if s == 2:
                            # off-chain: acc2T6 = acc_nat.T / 6 (via scaled
                            # identity), staged to SBUF so the s=0 STT has a
                            # single PSUM operand; zsum = zT + acc2T6
                            a26P = pa26.tile([HID, BL], F32, tag="a26",
                                             name="a26")
                            # regular matmul: transpose datapath would ignore
                            # the scaled identity's values
                            nc.tensor.matmul(
                                a26P[:], lhsT=acc_nat[:], rhs=id6S[:],
                                start=True, stop=True,
                            )
                            acc2T6 = zsp.tile([HID, BL], FP16, tag="a26s",
                                              name="a26s")
                            nc.vector.tensor_copy(acc2T6[:], a26P[:])
                            zsum = zsp.tile([HID, BL], F32R, tag="zsum",
                                            name="zsum")
                            nc.vector.tensor_tensor(
                                out=zsum[:], in0=zT, in1=acc2T6[:],
                                op=mybir.AluOpType.add,
                            )
                    else:
                        kt4P = pkt.tile([HID, BL], FP16, tag="kt", name="kt4")
                        nc.tensor.matmul(
                            kt4P[:], lhsT=kn[:], rhs=idS[:], is_transpose=True,
                            start=True, stop=True,
                        )
                        bt_cur = bt_next

            # final z update (last grid point)
            nc.vector.scalar_tensor_tensor(
                out=zT_sl(nstep),
                in0=kt4P[:],
                scalar=1.0 / 6.0,
                in1=zsum[:],
                op0=mybir.AluOpType.mult,
                op1=mybir.AluOpType.add,
            )
            nc.sync.dma_start(
                out=zs_out[:, nstep * BL : (nstep + 1) * BL],
                in_=zT_sl(nstep).bitcast(F32),
            )
    import sys

    print(f"[kernel] tile trace+schedule: {_time.time()-t0:.1f}s", file=sys.stderr)
    t1 = _time.time()
    nc.finalize()
    print(f"[kernel] finalize: {_time.time()-t1:.1f}s", file=sys.stderr)
    return nc


def _get_nc(nstep: int, with_b2: bool):
    key = (nstep, with_b2) + _flags()
    if key not in _CACHE:
        _CACHE[key] = _build(nstep, with_b2)
    return _CACHE[key]


def _host_prep(coeffs, Wi1, bi1, Wi2, bi2, W1, b1, W2, b2, nstep: int):
    coeffs = np.asarray(coeffs, dtype=np.float32)
    a = coeffs[:, :, 0:8]
    b = coeffs[:, :, 8:16]
    c = coeffs[:, :, 16:24]
    d = coeffs[:, :, 24:32]

    X0 = a[:, 0]
    z0 = np.tanh(
        np.maximum(X0 @ Wi1 + bi1, 0.0).astype(np.float32) @ Wi2 + bi2
    ).astype(np.float32)

    g = np.empty((B, nstep, 3, C_IN), dtype=np.float32)
    g[:, :, 0] = b[:, :nstep]
    g[:, :, 1] = 2.0 * b[:, :nstep] + 2.0 * c[:, :nstep] + 1.5 * d[:, :nstep]
    last = NSTEP - 1  # 126 in full problem
    for i in range(nstep):
        if i < last:
            g[:, i, 2] = b[:, i + 1]
        else:
            g[:, i, 2] = b[:, i] + 2.0 * c[:, i] + 3.0 * d[:, i]
    g16 = g.reshape(B, nstep * 3 * C_IN).astype(np.float16)

    tcols = np.empty((nstep, 3), dtype=np.float32)
    tcols[:, 0] = np.arange(nstep, dtype=np.float32)
    tcols[:, 1] = tcols[:, 0] + 0.5
    tcols[:, 2] = tcols[:, 0] + 1.0
    bias1 = (
        b1[None, None, :] + tcols[:, :, None] * W1[0][None, None, :]
    ).astype(np.float32)
    bias1 = bias1.reshape(nstep * 3, MLP_H).T.copy()  # [128, nstep*3]

    f16_big = _flags()[0]
    bias1t = np.ascontiguousarray(bias1.T.reshape(1, -1))
    shared = {
        "bias1": bias1,
        "bias1t": bias1t,
        "ones32": np.ones((1, BL), dtype=np.float32),
        "w1z": np.ascontiguousarray(
            W1[1:], dtype=(np.float16 if f16_big else np.float32)
        ),
        "w1zh": np.ascontiguousarray(W1[1:], dtype=np.float16),
        "w2": np.ascontiguousarray(W2, dtype=np.float16),
        "b2r": np.ascontiguousarray(b2[None, :], dtype=np.float16),
        "onesr": np.ones((1, BL), dtype=np.float16),
        "ident": np.eye(BL, dtype=np.float16),
        "ident6": (np.eye(BL) / 6.0).astype(np.float16),
    }
    in_maps = []
    for core in range(NCORES):
        sl = slice(core * BL, (core + 1) * BL)
        m = dict(shared)
        m["g"] = np.ascontiguousarray(g16[sl])
        m["z0t"] = np.ascontiguousarray(z0[sl].T)
        in_maps.append(m)
    return in_maps, z0


def kernel(coeffs, Wi1, bi1, Wi2, bi2, W1, b1, W2, b2, _nstep: int = NSTEP,
           _trace: bool = False):
    import time as _time
    import sys

    nstep = _nstep
    with_b2 = bool(np.any(np.asarray(b2)))
    nc = _get_nc(nstep, with_b2)
    in_maps, _ = _host_prep(
        coeffs, Wi1, bi1, Wi2, bi2, W1, b1, W2, b2, nstep
    )
    t0 = _time.time()
    res = run_bass_kernel_spmd(nc, in_maps, list(range(NCORES)), trace=_trace)
    print(f"[kernel] spmd run (compile+exec): {_time.time()-t0:.1f}s", file=sys.stderr)
    out = np.empty((B, nstep + 1, HID), dtype=np.float32)
    for core in range(NCORES):
        zs = res.results[core]["zs"].reshape(HID, nstep + 1, BL)
        out[core * BL : (core + 1) * BL] = zs.transpose(2, 1, 0)
    if _trace:
        kernel.last_results = res
    return out
